# revision 8
# baseline (speedup 1.0000x reference)
"""Trainium2 Bass kernel for nn_CosBlock (cos-attention transformer block).

Computation (B=4, T=2048, D=1024, H=16, Dh=64, Dmlp=4096), fp32:
    y  = LN1(x)
    q,k = tanh(y @ Wq|k) * pi/4 ; V = y @ Wv          (per head)
    cos-linear-attention via causal cumsum over T, normalized
    x2 = x + attn @ Wo
    out = x2 + gelu(LN2(x2) @ W1 + b1) @ W2 + b2

Distribution: tokens sharded over 8 cores (T split into 8 chunks of 256
per batch).  The only cross-core dependency is the cumsum carry: each
core AllGathers its per-128-block partial sums.  The gather is split
into one small collective per batch, fired as soon as that batch's two
token tiles finish phase B3, so the transfers complete underneath the
remaining B3 compute and phase D never waits.

Precision: attention matmuls in bf16 (fp32 PSUM accumulation), MLP in
fp8e4m3 with DoubleRow (2 fp8 weights per PE cell, 2x ALU rate).  W1 is
pre-scaled by 16 (descaled for free via gelu's input scale), W2 by 64
(descaled by a scalar-engine copy) so the small uniform weights stay in
fp8's normal range.  bf16/fp8 keep the PE fast-weight-load path enabled
(f32r disables it and roughly doubles per-matmul cost).  Transposes are
regular X.T@I matmuls (fp32 PSUM out), which also keeps the PE activity
monitor warm.  Cumsum is an upper-triangular matmul per 128-token block
with carry offsets folded into the same PSUM accumulation.
"""
from contextlib import ExitStack

import numpy as np

import concourse.bacc as bacc
import concourse.tile as tile
import concourse.mybir as mybir
from concourse import bass2jax

F32 = mybir.dt.float32
BF16 = mybir.dt.bfloat16
FP8 = mybir.dt.float8e4
DR = mybir.MatmulPerfMode.DoubleRow
AF = mybir.ActivationFunctionType
PI = float(np.pi)
LN_EPS = 1e-5
COS_EPS = 1e-6
W1S = 16.0   # host pre-scale on W1 (descaled inside gelu)
W2S = 64.0   # host pre-scale on W2/b2 (descaled by scalar copy)

NCORES = 8
B, T, D, H, DH, DMLP = 4, 2048, 1024, 16, 64, 4096
TC = T // NCORES          # tokens per core per batch = 256
NTOK = B * TC             # tokens per core = 1024
NTT = NTOK // 128         # token tiles per core = 8  (= B * 2 blocks)
NDK = D // 128            # 8
NDP = NDK // 2            # 4 dk pairs (fp8 DoubleRow)
NMT = DMLP // 128         # 32
NMP = NMT // 2            # 16 mt pairs
SCOLS = 2 * H * DH + 2 * H  # 2080 = cos*V | sin*V | cos_k | sin_k


def build_program(trivial_ln=True, repeats=1, n_devices=NCORES,
                  skip_collective=False, phase_marks=None):
    nc = bacc.Bacc("TRN2", target_bir_lowering=False, debug=False,
                   num_devices=n_devices)

    def din(name, shape, dt=BF16):
        return nc.dram_tensor(name, shape, dt, kind="ExternalInput").ap()

    xs_d = din("xs", [NTOK, D], F32)
    wqk_d = din("wqk", [D, 2 * H])
    wv_d = din("wv", [D, H * DH])
    wo_d = din("wo", [H * DH, D])
    w1_d = din("w1p8", [128, NMT, NDP, 2, 128], FP8)
    w2_d = din("w2p8", [128, NMP, 2, D], FP8)
    b1_d = din("b1r", [128, NMT], F32)
    b2_d = din("b2r", [1, D])
    u_d = din("utri", [128, 128])
    eye_d = din("eye", [128, 128])
    eye8_d = din("eye8", [128, 128], FP8)
    ones1_d = din("ones1", [1, 128])
    maskrep_d = din("maskrep", [2 * NCORES, 2 * 128])
    lnw_d = din("lnw", [4, D], F32)  # ln1_w, ln1_b, ln2_w, ln2_b rows
    out_d = nc.dram_tensor("out", [NTOK, D], F32, kind="ExternalOutput").ap()

    with tile.TileContext(nc) as tc, ExitStack() as top:
        consts = top.enter_context(tc.tile_pool(name="consts", bufs=1))
        u_sb = consts.tile([128, 128], BF16)
        eye_sb = consts.tile([128, 128], BF16)
        eye8 = consts.tile([128, 128], FP8)
        ones1 = consts.tile([1, 128], BF16)
        maskrep = consts.tile([2 * NCORES, 2 * 128], BF16)
        eps_t = consts.tile([128, 1], F32)
        halfpi = consts.tile([128, 1], F32)
        cose_t = consts.tile([128, 1], F32)
        b1_sb = consts.tile([128, NMT], F32)
        b2_sb = consts.tile([1, D], BF16)
        wqk_sb = consts.tile([128, NDK, 2 * H], BF16)
        # eye first on the sync queue (first transpose needs it), the
        # rest off-queue so the x-tile loads go out immediately
        nc.sync.dma_start(eye_sb[:], eye_d)
        nc.gpsimd.dma_start(u_sb[:], u_d)
        nc.gpsimd.dma_start(eye8[:], eye8_d)
        nc.gpsimd.dma_start(wqk_sb[:],
                            wqk_d.rearrange("(k p) n -> p k n", p=128))
        nc.gpsimd.dma_start(ones1[:], ones1_d)
        nc.gpsimd.dma_start(maskrep[:], maskrep_d)
        nc.gpsimd.dma_start(b1_sb[:], b1_d)
        nc.gpsimd.dma_start(b2_sb[:], b2_d)
        nc.vector.memset(eps_t[:], LN_EPS)
        nc.vector.memset(halfpi[:], PI / 2)
        nc.vector.memset(cose_t[:], COS_EPS)
        lnw_sb = None
        if not trivial_ln:
            lnw_sb = consts.tile([128, 4, D], F32)
            nc.sync.dma_start(
                lnw_sb[:], lnw_d[None, :, :].broadcast_to([128, 4, D]))

        for _rep in range(repeats):
            _body(nc, tc, trivial_ln, skip_collective, phase_marks,
                  xs_d, wv_d, wo_d, w1_d, w2_d, out_d,
                  u_sb, eye_sb, eye8, ones1, maskrep, eps_t,
                  halfpi, cose_t, b1_sb, b2_sb, wqk_sb, lnw_sb)

    nc.compile()
    return nc


def _layernorm(nc, eng, pool, x_t, y_t, eps_t, lnw_sb, widx):
    """token-major LN: y_t[128,1024] = LN(x_t).

    Stats stay on Vector (bn_stats is DVE-only); the wide normalize pass
    runs on `eng` (vector or gpsimd) to spread elementwise load."""
    stats = pool.tile([128, 6 * nc.vector.BN_STATS_DIM], F32, tag="ln_stats",
                      bufs=2)
    nsub = D // 512
    st3 = stats[:].rearrange("p (s d) -> p s d", s=6)
    xg = x_t[:].rearrange("p (s d) -> p s d", s=nsub)
    for s in range(nsub):
        nc.vector.bn_stats(out=st3[:, s, :], in_=xg[:, s, :])
    mv = pool.tile([128, nc.vector.BN_AGGR_DIM], F32, tag="ln_mv", bufs=2)
    nc.vector.bn_aggr(out=mv[:],
                      in_=stats[:, : nsub * nc.vector.BN_STATS_DIM]
                      .rearrange("p (s d) -> p s d", s=nsub))
    rstd = pool.tile([128, 1], F32, tag="ln_rstd", bufs=2)
    nc.scalar.activation(out=rstd[:], in_=mv[:, 1:2], func=AF.Sqrt,
                         bias=eps_t[:], scale=1.0)
    nc.vector.reciprocal(rstd[:], rstd[:])
    eng.tensor_scalar(
        out=y_t[:], in0=x_t[:], scalar1=mv[:, 0:1], scalar2=rstd[:],
        op0=mybir.AluOpType.subtract, op1=mybir.AluOpType.mult)
    if lnw_sb is not None:
        eng.tensor_mul(y_t[:], y_t[:], lnw_sb[:, widx, :])
        eng.tensor_add(y_t[:], y_t[:], lnw_sb[:, widx + 1, :])


def _transpose_into(nc, psp, dst_slices, src_t, eye_t, tag):
    """Transpose src_t[128, NDK*128] into dst_slices(dk) [128,128].

    Regular matmul X.T @ I (not transpose-mode): keeps FWL on and counts
    as PE activity for the HAM clock gate."""
    for dk in range(NDK):
        trp = psp.tile([128, 128], F32, tag=tag, bufs=2, name=f"trp_{tag}")
        nc.tensor.matmul(trp[:], src_t[:, dk * 128:(dk + 1) * 128],
                         eye_t[:], start=True, stop=True)
        if dk % 2 == 0:
            nc.vector.tensor_copy(dst_slices(dk), trp[:])
        else:
            nc.scalar.copy(out=dst_slices(dk), in_=trp[:])


def _mark(nc, phase_marks, name):
    if phase_marks is not None:
        phase_marks.append((name, nc.next_id()))


class _Scope:
    """Re-enterable named-scope helper: sc('name') opens, closing previous."""

    def __init__(self, nc):
        self.nc = nc
        self.cur = None

    def __call__(self, name):
        if self.cur is not None:
            self.nc.leave_named_scope(self.cur[0], self.cur[1], notify=False)
        self.cur = None
        if name is not None:
            sid, _ = self.nc.enter_named_scope(name, notify=False)
            self.cur = (name, sid)


def _body(nc, tc, trivial_ln, skip_collective, phase_marks, xs_d,
          wv_d, wo_d, w1_d, w2_d, out_d,
          u_sb, eye_sb, eye8, ones1, maskrep, eps_t, halfpi,
          cose_t, b1_sb, b2_sb, wqk_sb, lnw_sb):
    sc = _Scope(nc)
    with ExitStack() as ctx:
        # ---------- per-batch collective buffers ----------
        dram = ctx.enter_context(tc.tile_pool(name="dram", bufs=1,
                                              space="DRAM"))
        ag_ins = [dram.tile([2, SCOLS], BF16, name=f"agi{b}")
                  for b in range(B)]
        ag_outs = [dram.tile([NCORES, 2, SCOLS], BF16, name=f"ago{b}")
                   for b in range(B)]

        # persistent across D->E: x2 tiles + transposed LN2 output (fp8)
        de_pool = ctx.enter_context(tc.tile_pool(name="dep", bufs=1))
        x2ws = [de_pool.tile([128, D], F32, tag=f"x2w{tt}",
                             name=f"x2w{tt}") for tt in range(NTT)]
        y2T = [de_pool.tile([128, 2, NTOK], FP8, tag=f"y2T{dp}",
                            name=f"y2T{dp}") for dp in range(NDP)]

        rc_stack = ctx.enter_context(ExitStack())
        rc_pool = rc_stack.enter_context(tc.tile_pool(name="rcp", bufs=1))
        rc_ts = [rc_pool.tile([128, SCOLS], BF16, tag=f"rc{tt}",
                              name=f"rc{tt}") for tt in range(NTT)]
        qk_all = rc_pool.tile([128, NTT, 2 * H], F32, tag="qk_all")
        cos_all = rc_pool.tile([128, NTT, 2 * H], F32, tag="cos_all")
        sin_all = rc_pool.tile([128, NTT, 2 * H], F32, tag="sin_all")

        # ================= phase A (+B1 interleaved) =================
        with ExitStack() as pab:
            y1T_p = pab.enter_context(tc.tile_pool(name="y1T", bufs=1))
            y1T = [y1T_p.tile([128, NTOK], BF16, tag=f"y1T{dk}",
                              name=f"y1T{dk}") for dk in range(NDK)]
            work = pab.enter_context(tc.tile_pool(name="workA", bufs=3))
            wv_sb = y1T_p.tile([128, NDK, H * DH], BF16, tag="wv")

            _mark(nc, phase_marks, 'A_ln1')
            sc('A_ln1')
            with tc.tile_pool(name="psA", bufs=1, space="PSUM") as psA:
                for tt in range(NTT):
                    x_t = work.tile([128, D], F32, tag="x_t", bufs=3)
                    nc.sync.dma_start(
                        x_t[:], xs_d[tt * 128:(tt + 1) * 128, :])
                    y_t = work.tile([128, D], BF16, tag="y_t", bufs=3)
                    eng = nc.vector if tt % 2 == 0 else nc.gpsimd
                    _layernorm(nc, eng, work, x_t, y_t, eps_t, lnw_sb, 0)
                    _transpose_into(
                        nc, psA,
                        lambda dk, tt=tt: y1T[dk][:, tt * 128:(tt + 1) * 128],
                        y_t, eye_sb, "trA")
                    # fused B1: q/k projections for this tile
                    qk_ps = psA.tile([128, 2 * H], F32, tag="qk", bufs=2)
                    for dk in range(NDK):
                        nc.tensor.matmul(
                            qk_ps[:], y1T[dk][:, tt * 128:(tt + 1) * 128],
                            wqk_sb[:, dk, :],
                            start=(dk == 0), stop=(dk == NDK - 1))
                    nc.any.tensor_copy(qk_all[:, tt, :], qk_ps[:])

            # batched tanh / sin / cos
            nc.scalar.activation(out=qk_all[:], in_=qk_all[:], func=AF.Tanh)
            nc.scalar.activation(out=sin_all[:], in_=qk_all[:], func=AF.Sin,
                                 scale=PI / 4)
            nc.scalar.activation(out=cos_all[:], in_=qk_all[:], func=AF.Sin,
                                 scale=PI / 4, bias=halfpi[:])

            for dk in range(NDK):
                nc.sync.dma_start(
                    wv_sb[:, dk, :],
                    wv_d[dk * 128:(dk + 1) * 128, :])
            _mark(nc, phase_marks, 'B3_V_S_C')
            sc('B3_V_S_C')
            psB = pab.enter_context(
                tc.tile_pool(name="psB", bufs=1, space="PSUM"))
            for tt in range(NTT):
                v_ps = psB.tile([128, H * DH], F32, tag="v", bufs=2)
                for dk in range(NDK):
                    for nh in range(2):
                        nc.tensor.matmul(
                            v_ps[:, nh * 512:(nh + 1) * 512],
                            y1T[dk][:, tt * 128:(tt + 1) * 128],
                            wv_sb[:, dk, nh * 512:(nh + 1) * 512],
                            start=(dk == 0), stop=(dk == NDK - 1))
                s_t = work.tile([128, SCOLS], BF16, tag="s_t", bufs=2)
                v3 = v_ps[:].rearrange("p (h d) -> p h d", h=H)
                nc.vector.tensor_mul(
                    s_t[:, 0:H * DH].rearrange("p (h d) -> p h d", h=H),
                    v3,
                    cos_all[:, tt, H:2 * H][:, :, None]
                    .broadcast_to([128, H, DH]))
                nc.vector.tensor_mul(
                    s_t[:, H * DH:2 * H * DH]
                    .rearrange("p (h d) -> p h d", h=H),
                    v3,
                    sin_all[:, tt, H:2 * H][:, :, None]
                    .broadcast_to([128, H, DH]))
                nc.any.tensor_copy(s_t[:, 2 * H * DH:2 * H * DH + H],
                                   cos_all[:, tt, H:2 * H])
                nc.any.tensor_copy(s_t[:, 2 * H * DH + H:SCOLS],
                                   sin_all[:, tt, H:2 * H])
                # raw causal cumsum of S (U-matmul) into resident rc;
                # row 127 = block total feeds the carry exchange
                b, j = tt // 2, tt % 2
                for c0 in range(0, SCOLS, 1024):
                    cw = min(1024, SCOLS - c0)
                    cum = psB.tile([128, 1024], F32, tag="cum", bufs=2)
                    for cc in range(0, cw, 512):
                        ccw = min(512, cw - cc)
                        nc.tensor.matmul(
                            cum[:, cc:cc + ccw], u_sb[:],
                            s_t[:, c0 + cc:c0 + cc + ccw],
                            start=True, stop=True)
                    nc.scalar.copy(out=rc_ts[tt][:, c0:c0 + cw],
                                   in_=cum[:, :cw])
                    nc.sync.dma_start(
                        ag_ins[b][j:j + 1, c0:c0 + cw],
                        rc_ts[tt][127:128, c0:c0 + cw])
                # fire this batch's tiny AllGather as soon as both its
                # tiles are done -- transfers hide under remaining B3
                if j == 1:
                    if skip_collective:
                        nc.gpsimd.dma_start(ag_outs[b][0], ag_ins[b][:])
                    else:
                        nc.gpsimd.collective_compute(
                            "AllGather", mybir.AluOpType.bypass,
                            replica_groups=[list(range(NCORES))],
                            ins=[ag_ins[b].opt()], outs=[ag_outs[b].opt()])

        # ========== phase D: attention + residual + LN2 (per batch) =====
        _mark(nc, phase_marks, 'D_attn')
        sc('D_attn')
        with ExitStack() as pd:
            work = pd.enter_context(tc.tile_pool(name="workD", bufs=3))
            wo_pool = pd.enter_context(tc.tile_pool(name="wop", bufs=1))
            wo_sb = wo_pool.tile([128, NDK, D], BF16, tag="wo")
            for dk in range(NDK):
                nc.sync.dma_start(
                    wo_sb[:, dk, :],
                    wo_d[dk * 128:(dk + 1) * 128, :])

            den_all = wo_pool.tile([128, NTT, H], F32, tag="den_all")
            rqc_all = wo_pool.tile([128, NTT, H], F32, tag="rqc_all")
            rqs_all = wo_pool.tile([128, NTT, H], F32, tag="rqs_all")
            psD = pd.enter_context(
                tc.tile_pool(name="psD", bufs=1, space="PSUM"))
            for b in range(B):
                t0 = 2 * b
                # --- D1: scalar-sum carries + denominators for batch b ---
                csc = psD.tile([128, 2, 2 * H], F32, tag="csc", bufs=1)
                gsc = work.tile([2 * NCORES, 2 * H], BF16, tag="gsc", bufs=2)
                nc.sync.dma_start(
                    gsc[:], ag_outs[b][:, :, 2 * H * DH:SCOLS])
                for j in range(2):
                    nc.tensor.matmul(csc[:, j, :],
                                     maskrep[:, j * 128:(j + 1) * 128],
                                     gsc[:], start=True, stop=False)
                    nc.tensor.matmul(csc[:, j, :], eye_sb[:],
                                     rc_ts[t0 + j][:, 2 * H * DH:SCOLS],
                                     start=False, stop=True)
                den = den_all[:, t0:t0 + 2, :]
                t2 = work.tile([128, 2, H], F32, tag="t2", bufs=2)
                nc.vector.tensor_mul(den, csc[:, :, 0:H],
                                     cos_all[:, t0:t0 + 2, 0:H])
                nc.vector.tensor_mul(t2[:], csc[:, :, H:2 * H],
                                     sin_all[:, t0:t0 + 2, 0:H])
                nc.vector.tensor_add(den, den, t2[:])
                nc.vector.tensor_scalar(
                    out=den, in0=den, scalar1=cose_t[:],
                    scalar2=None, op0=mybir.AluOpType.add)
                nc.vector.reciprocal(den, den)
                nc.vector.tensor_mul(rqc_all[:, t0:t0 + 2, :], den,
                                     cos_all[:, t0:t0 + 2, 0:H])
                nc.vector.tensor_mul(rqs_all[:, t0:t0 + 2, :], den,
                                     sin_all[:, t0:t0 + 2, 0:H])

                # --- D2: heads, Wo, residual, LN2 for tiles 2b, 2b+1 ---
                for j in range(2):
                    tt = t0 + j
                    rc_t = rc_ts[tt]
                    gath = work.tile([2 * NCORES, 2 * H * DH], BF16,
                                     tag="gath", bufs=2)
                    nc.sync.dma_start(
                        gath[:], ag_outs[b][:, :, 0:2 * H * DH])

                    h_t = work.tile([128, H * DH], BF16, tag="h_t", bufs=2)
                    tmpc = work.tile([128, H * DH], BF16, tag="tmpc", bufs=2)
                    for half, rqa in ((0, rqc_all), (1, rqs_all)):
                        base = half * H * DH
                        dst = tmpc if half == 0 else h_t
                        for c0 in range(0, H * DH, 512):
                            cv = psD.tile([128, 512], F32, tag="cumv",
                                          bufs=2)
                            nc.tensor.matmul(
                                cv[:],
                                maskrep[:, j * 128:(j + 1) * 128],
                                gath[:, base + c0:base + c0 + 512],
                                start=True, stop=False)
                            nc.tensor.matmul(
                                cv[:], eye_sb[:],
                                rc_t[:, base + c0:base + c0 + 512],
                                start=False, stop=True)
                            nc.vector.tensor_mul(
                                dst[:, c0:c0 + 512]
                                .rearrange("p (h d) -> p h d", h=H // 2),
                                cv[:].rearrange("p (h d) -> p h d", h=H // 2),
                                rqa[:, tt, c0 // DH:(c0 + 512) // DH]
                                [:, :, None].broadcast_to([128, H // 2, DH]))

                    # transpose heads (summing both halves in PSUM) + Wo
                    x_t = work.tile([128, D], F32, tag="x_t2", bufs=2)
                    nc.sync.dma_start(x_t[:],
                                      xs_d[tt * 128:(tt + 1) * 128, :])
                    attn = psD.tile([128, D], F32, tag="attn", bufs=1)
                    for dk in range(NDK):
                        trp = psD.tile([128, 128], F32, tag="trD", bufs=2)
                        nc.tensor.matmul(
                            trp[:], tmpc[:, dk * 128:(dk + 1) * 128],
                            eye_sb[:], start=True, stop=False)
                        nc.tensor.matmul(
                            trp[:], h_t[:, dk * 128:(dk + 1) * 128],
                            eye_sb[:], start=False, stop=True)
                        hT = work.tile([128, 128], BF16, tag="hT", bufs=2)
                        nc.any.tensor_copy(hT[:], trp[:])
                        for nh in range(2):
                            nc.tensor.matmul(
                                attn[:, nh * 512:(nh + 1) * 512], hT[:],
                                wo_sb[:, dk, nh * 512:(nh + 1) * 512],
                                start=(dk == 0), stop=(dk == NDK - 1))
                    # residual add on DVE (PSUM + SBUF -> SBUF)
                    nc.vector.tensor_add(x2ws[tt][:], attn[:], x_t[:])

                    # LN2 + transpose into resident fp8 y2T (normalize on
                    # gpsimd -- vector is busy with the rq multiplies)
                    y_t = work.tile([128, D], FP8, tag="y2_t", bufs=2)
                    _layernorm(nc, nc.gpsimd, work, x2ws[tt], y_t, eps_t,
                               lnw_sb, 2)
                    for dk in range(NDK):
                        trp = psD.tile([128, 128], F32, tag="trD", bufs=2)
                        nc.tensor.matmul(trp[:],
                                         y_t[:, dk * 128:(dk + 1) * 128],
                                         eye8[:], start=True, stop=True)
                        dst = y2T[dk // 2][:, dk % 2,
                                           tt * 128:(tt + 1) * 128]
                        if dk % 2 == 0:
                            nc.vector.tensor_copy(dst, trp[:])
                        else:
                            nc.scalar.copy(out=dst, in_=trp[:])

        rc_stack.close()

        # ================= phase E: MLP (fp8 DoubleRow) =================
        _mark(nc, phase_marks, 'E_mlp')
        sc('E_mlp')
        with ExitStack() as pe:
            wpool = pe.enter_context(tc.tile_pool(name="wmlp", bufs=3))
            h1_pool = pe.enter_context(tc.tile_pool(name="h1p", bufs=1))
            opool = pe.enter_context(tc.tile_pool(name="outp", bufs=3))
            h1 = h1_pool.tile([128, NMP, 2, NTOK], FP8, tag="h1")
            with tc.tile_pool(name="psE1", bufs=1, space="PSUM") as psE1:
                for mt in range(NMT):
                    w1_t = wpool.tile([128, NDP, 2, 128], FP8, tag="w1t")
                    nc.sync.dma_start(w1_t[:], w1_d[:, mt])
                    h1ps = psE1.tile([128, NTOK], F32, tag="h1ps", bufs=2)
                    for dp in range(NDP):
                        for nh in range(2):
                            nc.tensor.matmul(
                                h1ps[:, nh * 512:(nh + 1) * 512],
                                w1_t[:, dp], y2T[dp][:, :,
                                                     nh * 512:(nh + 1) * 512],
                                start=(dp == 0), stop=(dp == NDP - 1),
                                perf_mode=DR)
                    nc.scalar.activation(
                        out=h1[:, mt // 2, mt % 2, :], in_=h1ps[:],
                        func=AF.Gelu_apprx_tanh,
                        bias=b1_sb[:, mt:mt + 1], scale=1.0 / W1S)
            _mark(nc, phase_marks, 'E2_mlp2')
            sc('E2_mlp2')
            with tc.tile_pool(name="psE2", bufs=1, space="PSUM") as psE2:
                for dhalf in range(2):
                    d0 = dhalf * 512
                    ops = [psE2.tile([128, 512], F32, tag=f"o{tt}",
                                     name=f"ops{tt}") for tt in range(NTT)]
                    for mp in range(NMP):
                        w2_t = wpool.tile([128, 2, 512], FP8, tag="w2t",
                                          bufs=4)
                        nc.sync.dma_start(
                            w2_t[:], w2_d[:, mp, :, d0:d0 + 512])
                        for tt in range(NTT):
                            nc.tensor.matmul(
                                ops[tt][:],
                                h1[:, mp, :, tt * 128:(tt + 1) * 128],
                                w2_t[:],
                                start=(mp == 0), stop=False, perf_mode=DR)
                    for tt in range(NTT):
                        nc.tensor.matmul(ops[tt][:], ones1[:],
                                         b2_sb[:, d0:d0 + 512],
                                         start=False, stop=True)
                        # descale W2S on the (idle) scalar engine
                        osc = opool.tile([128, 512], F32, tag="osc")
                        nc.scalar.activation(out=osc[:], in_=ops[tt][:],
                                             func=AF.Copy, scale=1.0 / W2S)
                        o_t = opool.tile([128, 512], F32, tag="o_t")
                        nc.vector.tensor_add(o_t[:], osc[:],
                                             x2ws[tt][:, d0:d0 + 512])
                        nc.sync.dma_start(
                            out_d[tt * 128:(tt + 1) * 128, d0:d0 + 512],
                            o_t[:])
        sc(None)


# ---------------------------------------------------------------------------
# host side
# ---------------------------------------------------------------------------

def _prep_inputs(x, W_Q, W_K, W_V, W_O, ln1_w, ln1_b, ln2_w, ln2_b,
                 W1, b1, W2, b2):
    import ml_dtypes
    f = np.float32
    bf = ml_dtypes.bfloat16
    f8 = ml_dtypes.float8_e4m3
    wqk = np.concatenate(
        [np.asarray(W_Q)[:, :, 0].T, np.asarray(W_K)[:, :, 0].T],
        axis=1).astype(bf)                                       # [D, 2H]
    wv = np.ascontiguousarray(
        np.asarray(W_V).transpose(1, 0, 2).reshape(D, H * DH)).astype(bf)
    wo = np.ascontiguousarray(
        np.asarray(W_O).transpose(2, 1, 0).reshape(H * DH, D)).astype(bf)
    # w1 pre-tiled/paired: w1p8[p, mt, dp, i, c] = W1S*W1[(2dp+i)*128+p, ...]
    w1p8 = np.ascontiguousarray(
        (np.asarray(W1, f) * W1S).reshape(NDP, 2, 128, NMT, 128)
        .transpose(2, 3, 0, 1, 4)).astype(f8)        # [128, NMT, NDP, 2, 128]
    # w2 paired: w2p8[p, mp, i, d] = W2S*W2[(2mp+i)*128+p, d]
    w2p8 = np.ascontiguousarray(
        (np.asarray(W2, f) * W2S).reshape(NMP, 2, 128, D)
        .transpose(2, 0, 1, 3)).astype(f8)           # [128, NMP, 2, D]
    b1r = np.ascontiguousarray(np.asarray(b1, f).reshape(NMT, 128).T)
    b2r = (np.asarray(b2, f) * W2S).reshape(1, D).astype(bf)
    utri = np.triu(np.ones((128, 128), f)).astype(bf)
    eye = np.eye(128, dtype=f).astype(bf)
    eye8 = np.eye(128, dtype=f).astype(f8)
    ones1 = np.ones((1, 128), bf)
    lnw = np.stack([np.asarray(ln1_w, f), np.asarray(ln1_b, f),
                    np.asarray(ln2_w, f), np.asarray(ln2_b, f)])
    common = dict(wqk=wqk, wv=wv, wo=wo,
                  w1p8=w1p8, w2p8=w2p8,
                  b1r=b1r, b2r=b2r, utri=utri, eye=eye, eye8=eye8,
                  ones1=ones1, lnw=lnw)
    x = np.asarray(x, f)
    in_maps = []
    for c in range(NCORES):
        xs = np.ascontiguousarray(
            x[:, c * TC:(c + 1) * TC, :].reshape(NTOK, D))
        masks = np.zeros((2 * NCORES, 2), f)
        for cp in range(NCORES):
            for jp in range(2):
                row = 2 * cp + jp
                masks[row, 0] = 1.0 if cp < c else 0.0
                masks[row, 1] = 1.0 if (cp < c or (cp == c and jp == 0)) \
                    else 0.0
        maskrep = np.concatenate(
            [np.repeat(masks[:, jj:jj + 1], 128, axis=1) for jj in range(2)],
            axis=1).astype(bf)
        in_maps.append(dict(common, xs=xs, maskrep=maskrep))
    trivial = bool(np.allclose(ln1_w, 1) and np.allclose(ln2_w, 1)
                   and np.allclose(ln1_b, 0) and np.allclose(ln2_b, 0))
    return in_maps, trivial


_CACHE = {}


def make_runner(nc):
    """Build a reusable jitted callable for this compiled Bass program."""
    import jax
    from jax.sharding import Mesh, PartitionSpec
    from jax.experimental.shard_map import shard_map

    bass2jax.install_neuronx_cc_hook()
    partition_name = (nc.partition_id_tensor.name
                      if nc.partition_id_tensor else None)
    in_names, out_names, out_avals, zero_outs = [], [], [], []
    for alloc in nc.m.functions[0].allocations:
        if not isinstance(alloc, mybir.MemoryLocationSet):
            continue
        name = alloc.memorylocations[0].name
        if alloc.kind == "ExternalInput":
            if name != partition_name:
                in_names.append(name)
        elif alloc.kind == "ExternalOutput":
            out_names.append(name)
            shape = tuple(alloc.tensor_shape)
            dtype = mybir.dt.np(alloc.dtype)
            out_avals.append(jax.core.ShapedArray(shape, dtype))
            zero_outs.append(np.zeros(shape, dtype))
    n_params = len(in_names)
    n_outs = len(out_avals)
    in_names_all = in_names + out_names
    if partition_name is not None:
        in_names_all.append(partition_name)

    def _bodyfn(*args):
        operands = list(args)
        if partition_name is not None:
            operands.append(bass2jax.partition_id_tensor())
        outs = bass2jax._bass_exec_p.bind(
            *operands,
            out_avals=tuple(out_avals),
            in_names=tuple(in_names_all),
            out_names=tuple(out_names),
            lowering_input_output_aliases=(),
            sim_require_finite=True,
            sim_require_nnan=True,
            nc=nc,
        )
        return tuple(outs)

    devices = jax.devices()[:NCORES]
    mesh = Mesh(np.asarray(devices), ("core",))
    sharded = jax.jit(
        shard_map(_bodyfn, mesh=mesh,
                  in_specs=(PartitionSpec("core"),) * (n_params + n_outs),
                  out_specs=(PartitionSpec("core"),) * n_outs,
                  check_rep=False),
        keep_unused=True)

    def run(in_maps):
        concat_in = [
            np.concatenate([np.asarray(m[name]) for m in in_maps], axis=0)
            for name in in_names
        ]
        zeros = [np.zeros((NCORES * z.shape[0], *z.shape[1:]), z.dtype)
                 for z in zero_outs]
        outs = sharded(*concat_in, *zeros)
        jax.block_until_ready(outs)
        return {
            name: np.asarray(outs[i]).reshape(NCORES, *out_avals[i].shape)
            for i, name in enumerate(out_names)
        }

    return run


def kernel(**inputs):
    in_maps, trivial = _prep_inputs(**inputs)
    key = ("prog", trivial)
    if key not in _CACHE:
        _CACHE[key] = build_program(trivial_ln=trivial)
    nc = _CACHE[key]
    rkey = ("run", trivial)
    if rkey not in _CACHE:
        _CACHE[rkey] = make_runner(nc)
    outs = _CACHE[rkey](in_maps)
    out = outs["out"]  # [NCORES, NTOK, D]
    res = np.empty((B, T, D), np.float32)
    for c in range(NCORES):
        res[:, c * TC:(c + 1) * TC, :] = out[c].reshape(B, TC, D)
    return res


# revision 9
# speedup vs baseline: 1.4421x; 1.4421x over previous
"""Trainium2 Bass kernel for nn_CosBlock (cos-attention transformer block).

Computation (B=4, T=2048, D=1024, H=16, Dh=64, Dmlp=4096), fp32:
    y  = LN1(x)
    q,k = tanh(y @ Wq|k) * pi/4 ; V = y @ Wv          (per head)
    cos-linear-attention via causal cumsum over T, normalized
    x2 = x + attn @ Wo
    out = x2 + gelu(LN2(x2) @ W1 + b1) @ W2 + b2

Distribution: tokens sharded over 8 cores (T split into 8 chunks of 256
per batch).  The only cross-core dependency is the cumsum carry: each
core AllGathers its per-128-block partial sums.  The gather is split
into one small collective per batch, fired as soon as that batch's two
token tiles finish phase B3, so the transfers complete underneath the
remaining B3 compute and phase D never waits.

Precision: attention matmuls in bf16 (fp32 PSUM accumulation), MLP in
fp8e4m3 with DoubleRow (2 fp8 weights per PE cell, 2x ALU rate).  W1 is
pre-scaled by 16 (descaled for free via gelu's input scale), W2 by 64
(descaled by a scalar-engine copy) so the small uniform weights stay in
fp8's normal range.  bf16/fp8 keep the PE fast-weight-load path enabled
(f32r disables it and roughly doubles per-matmul cost).  Transposes are
regular X.T@I matmuls (fp32 PSUM out), which also keeps the PE activity
monitor warm.  Cumsum is an upper-triangular matmul per 128-token block
with carry offsets folded into the same PSUM accumulation.
"""
from contextlib import ExitStack

import numpy as np

import concourse.bacc as bacc
import concourse.tile as tile
import concourse.mybir as mybir
from concourse import bass2jax

F32 = mybir.dt.float32
BF16 = mybir.dt.bfloat16
FP8 = mybir.dt.float8e4
DR = mybir.MatmulPerfMode.DoubleRow
AF = mybir.ActivationFunctionType
PI = float(np.pi)
LN_EPS = 1e-5
COS_EPS = 1e-6
W1S = 16.0   # host pre-scale on W1 (descaled inside gelu)
W2S = 64.0   # host pre-scale on W2/b2 (descaled by scalar copy)

NCORES = 8
B, T, D, H, DH, DMLP = 4, 2048, 1024, 16, 64, 4096
TC = T // NCORES          # tokens per core per batch = 256
NTOK = B * TC             # tokens per core = 1024
NTT = NTOK // 128         # token tiles per core = 8  (= B * 2 blocks)
NDK = D // 128            # 8
NDP = NDK // 2            # 4 dk pairs (fp8 DoubleRow)
NMT = DMLP // 128         # 32
NMP = NMT // 2            # 16 mt pairs
SCOLS = 2 * H * DH + 2 * H  # 2080 = cos*V | sin*V | cos_k | sin_k


def build_program(trivial_ln=True, repeats=1, n_devices=NCORES,
                  skip_collective=False, phase_marks=None):
    nc = bacc.Bacc("TRN2", target_bir_lowering=False, debug=False,
                   num_devices=n_devices)

    def din(name, shape, dt=BF16):
        return nc.dram_tensor(name, shape, dt, kind="ExternalInput").ap()

    xs_d = din("xs", [NTOK, D], F32)
    wqk_d = din("wqk", [D, 2 * H])
    wv_d = din("wv", [D, H * DH])
    wo_d = din("wo", [H * DH, D])
    w1_d = din("w1p8", [128, NMT, NDP, 2, 128], FP8)
    w2_d = din("w2p8", [128, NMP, 2, D], FP8)
    b1_d = din("b1r", [128, NMT], F32)
    b2_d = din("b2r", [1, D])
    u_d = din("utri", [128, 128])
    eye_d = din("eye", [128, 128])
    eye8_d = din("eye8", [128, 128], FP8)
    ones1_d = din("ones1", [1, 128])
    maskrep_d = din("maskrep", [2 * NCORES, 2 * 128])
    lnw_d = din("lnw", [4, D], F32)  # ln1_w, ln1_b, ln2_w, ln2_b rows
    out_d = nc.dram_tensor("out", [NTOK, D], F32, kind="ExternalOutput").ap()

    with tile.TileContext(nc) as tc, ExitStack() as top:
        consts = top.enter_context(tc.tile_pool(name="consts", bufs=1))
        u_sb = consts.tile([128, 128], BF16)
        eye_sb = consts.tile([128, 128], BF16)
        eye8 = consts.tile([128, 128], FP8)
        ones1 = consts.tile([1, 128], BF16)
        maskrep = consts.tile([2 * NCORES, 2 * 128], BF16)
        eps_t = consts.tile([128, 1], F32)
        halfpi = consts.tile([128, 1], F32)
        cose_t = consts.tile([128, 1], F32)
        b1_sb = consts.tile([128, NMT], F32)
        b2_sb = consts.tile([1, D], BF16)
        wqk_sb = consts.tile([128, NDK, 2 * H], BF16)
        # eye first on the sync queue (first transpose needs it), the
        # rest off-queue so the x-tile loads go out immediately
        nc.sync.dma_start(eye_sb[:], eye_d)
        nc.gpsimd.dma_start(u_sb[:], u_d)
        nc.gpsimd.dma_start(eye8[:], eye8_d)
        nc.gpsimd.dma_start(wqk_sb[:],
                            wqk_d.rearrange("(k p) n -> p k n", p=128))
        nc.gpsimd.dma_start(ones1[:], ones1_d)
        nc.gpsimd.dma_start(maskrep[:], maskrep_d)
        nc.gpsimd.dma_start(b1_sb[:], b1_d)
        nc.gpsimd.dma_start(b2_sb[:], b2_d)
        nc.vector.memset(eps_t[:], LN_EPS)
        nc.vector.memset(halfpi[:], PI / 2)
        nc.vector.memset(cose_t[:], COS_EPS)
        lnw_sb = None
        if not trivial_ln:
            lnw_sb = consts.tile([128, 4, D], F32)
            nc.sync.dma_start(
                lnw_sb[:], lnw_d[None, :, :].broadcast_to([128, 4, D]))

        for _rep in range(repeats):
            _body(nc, tc, trivial_ln, skip_collective, phase_marks,
                  xs_d, wv_d, wo_d, w1_d, w2_d, out_d,
                  u_sb, eye_sb, eye8, ones1, maskrep, eps_t,
                  halfpi, cose_t, b1_sb, b2_sb, wqk_sb, lnw_sb)

    nc.compile()
    return nc


def _layernorm(nc, eng, pool, x_t, y_t, eps_t, lnw_sb, widx):
    """token-major LN: y_t[128,1024] = LN(x_t).

    Stats stay on Vector (bn_stats is DVE-only); the wide normalize pass
    runs on `eng` (vector or gpsimd) to spread elementwise load."""
    stats = pool.tile([128, 6 * nc.vector.BN_STATS_DIM], F32, tag="ln_stats",
                      bufs=2)
    nsub = D // 512
    st3 = stats[:].rearrange("p (s d) -> p s d", s=6)
    xg = x_t[:].rearrange("p (s d) -> p s d", s=nsub)
    for s in range(nsub):
        nc.vector.bn_stats(out=st3[:, s, :], in_=xg[:, s, :])
    mv = pool.tile([128, nc.vector.BN_AGGR_DIM], F32, tag="ln_mv", bufs=2)
    nc.vector.bn_aggr(out=mv[:],
                      in_=stats[:, : nsub * nc.vector.BN_STATS_DIM]
                      .rearrange("p (s d) -> p s d", s=nsub))
    rstd = pool.tile([128, 1], F32, tag="ln_rstd", bufs=2)
    nc.scalar.activation(out=rstd[:], in_=mv[:, 1:2], func=AF.Sqrt,
                         bias=eps_t[:], scale=1.0)
    nc.vector.reciprocal(rstd[:], rstd[:])
    eng.tensor_scalar(
        out=y_t[:], in0=x_t[:], scalar1=mv[:, 0:1], scalar2=rstd[:],
        op0=mybir.AluOpType.subtract, op1=mybir.AluOpType.mult)
    if lnw_sb is not None:
        eng.tensor_mul(y_t[:], y_t[:], lnw_sb[:, widx, :])
        eng.tensor_add(y_t[:], y_t[:], lnw_sb[:, widx + 1, :])


def _transpose_into(nc, psp, dst_slices, src_t, eye_t, tag):
    """Transpose src_t[128, NDK*128] into dst_slices(dk) [128,128].

    Regular matmul X.T @ I (not transpose-mode): keeps FWL on and counts
    as PE activity for the HAM clock gate."""
    for dk in range(NDK):
        trp = psp.tile([128, 128], F32, tag=tag, bufs=2, name=f"trp_{tag}")
        nc.tensor.matmul(trp[:], src_t[:, dk * 128:(dk + 1) * 128],
                         eye_t[:], start=True, stop=True)
        if dk % 2 == 0:
            nc.vector.tensor_copy(dst_slices(dk), trp[:])
        else:
            nc.scalar.copy(out=dst_slices(dk), in_=trp[:])


def _mark(nc, phase_marks, name):
    if phase_marks is not None:
        phase_marks.append((name, nc.next_id()))


class _Scope:
    """Re-enterable named-scope helper: sc('name') opens, closing previous."""

    def __init__(self, nc):
        self.nc = nc
        self.cur = None

    def __call__(self, name):
        if self.cur is not None:
            self.nc.leave_named_scope(self.cur[0], self.cur[1], notify=False)
        self.cur = None
        if name is not None:
            sid, _ = self.nc.enter_named_scope(name, notify=False)
            self.cur = (name, sid)


def _body(nc, tc, trivial_ln, skip_collective, phase_marks, xs_d,
          wv_d, wo_d, w1_d, w2_d, out_d,
          u_sb, eye_sb, eye8, ones1, maskrep, eps_t, halfpi,
          cose_t, b1_sb, b2_sb, wqk_sb, lnw_sb):
    sc = _Scope(nc)
    with ExitStack() as ctx:
        # ---------- per-batch collective buffers ----------
        dram = ctx.enter_context(tc.tile_pool(name="dram", bufs=1,
                                              space="DRAM"))
        ag_ins = [dram.tile([2, SCOLS], BF16, name=f"agi{b}")
                  for b in range(B)]
        ag_outs = [dram.tile([NCORES, 2, SCOLS], BF16, name=f"ago{b}")
                   for b in range(B)]

        # persistent across D->E: x2 tiles + transposed LN2 output (fp8)
        de_pool = ctx.enter_context(tc.tile_pool(name="dep", bufs=1))
        wo_sb = de_pool.tile([128, NDK, D], BF16, tag="wo")
        x2ws = [de_pool.tile([128, D], F32, tag=f"x2w{tt}",
                             name=f"x2w{tt}") for tt in range(NTT)]
        y2T = [de_pool.tile([128, 2, NTOK], FP8, tag=f"y2T{dp}",
                            name=f"y2T{dp}") for dp in range(NDP)]

        rc_stack = ctx.enter_context(ExitStack())
        rc_pool = rc_stack.enter_context(tc.tile_pool(name="rcp", bufs=1))
        rc_ts = [rc_pool.tile([128, SCOLS], BF16, tag=f"rc{tt}",
                              name=f"rc{tt}") for tt in range(NTT)]
        qk_all = rc_pool.tile([128, NTT, 2 * H], F32, tag="qk_all")
        cos_all = rc_pool.tile([128, NTT, 2 * H], F32, tag="cos_all")
        sin_all = rc_pool.tile([128, NTT, 2 * H], F32, tag="sin_all")

        # ================= phase A (+B1 interleaved) =================
        with ExitStack() as pab:
            y1T_p = pab.enter_context(tc.tile_pool(name="y1T", bufs=1))
            y1T = [y1T_p.tile([128, NTOK], BF16, tag=f"y1T{dk}",
                              name=f"y1T{dk}") for dk in range(NDK)]
            work = pab.enter_context(tc.tile_pool(name="workA", bufs=3))
            wv_sb = y1T_p.tile([128, NDK, H * DH], BF16, tag="wv")

            _mark(nc, phase_marks, 'A_ln1')
            sc('A_ln1')
            with tc.tile_pool(name="psA", bufs=1, space="PSUM") as psA:
                for tt in range(NTT):
                    x_t = work.tile([128, D], F32, tag="x_t", bufs=3)
                    nc.sync.dma_start(
                        x_t[:], xs_d[tt * 128:(tt + 1) * 128, :])
                    y_t = work.tile([128, D], BF16, tag="y_t", bufs=3)
                    _layernorm(nc, nc.vector, work, x_t, y_t, eps_t,
                               lnw_sb, 0)
                    _transpose_into(
                        nc, psA,
                        lambda dk, tt=tt: y1T[dk][:, tt * 128:(tt + 1) * 128],
                        y_t, eye_sb, "trA")
                    # fused B1: q/k projections for this tile
                    qk_ps = psA.tile([128, 2 * H], F32, tag="qk", bufs=2)
                    for dk in range(NDK):
                        nc.tensor.matmul(
                            qk_ps[:], y1T[dk][:, tt * 128:(tt + 1) * 128],
                            wqk_sb[:, dk, :],
                            start=(dk == 0), stop=(dk == NDK - 1))
                    nc.any.tensor_copy(qk_all[:, tt, :], qk_ps[:])

            # batched tanh / sin / cos
            nc.scalar.activation(out=qk_all[:], in_=qk_all[:], func=AF.Tanh)
            nc.scalar.activation(out=sin_all[:], in_=qk_all[:], func=AF.Sin,
                                 scale=PI / 4)
            nc.scalar.activation(out=cos_all[:], in_=qk_all[:], func=AF.Sin,
                                 scale=PI / 4, bias=halfpi[:])

            for dk in range(NDK):
                nc.sync.dma_start(
                    wv_sb[:, dk, :],
                    wv_d[dk * 128:(dk + 1) * 128, :])
            for dk in range(NDK):
                nc.sync.dma_start(
                    wo_sb[:, dk, :],
                    wo_d[dk * 128:(dk + 1) * 128, :])
            _mark(nc, phase_marks, 'B3_V_S_C')
            sc('B3_V_S_C')
            psB = pab.enter_context(
                tc.tile_pool(name="psB", bufs=1, space="PSUM"))
            for tt in range(NTT):
                v_ps = psB.tile([128, H * DH], F32, tag="v", bufs=2)
                for dk in range(NDK):
                    for nh in range(2):
                        nc.tensor.matmul(
                            v_ps[:, nh * 512:(nh + 1) * 512],
                            y1T[dk][:, tt * 128:(tt + 1) * 128],
                            wv_sb[:, dk, nh * 512:(nh + 1) * 512],
                            start=(dk == 0), stop=(dk == NDK - 1))
                s_t = work.tile([128, SCOLS], BF16, tag="s_t", bufs=2)
                v3 = v_ps[:].rearrange("p (h d) -> p h d", h=H)
                nc.vector.tensor_mul(
                    s_t[:, 0:H * DH].rearrange("p (h d) -> p h d", h=H),
                    v3,
                    cos_all[:, tt, H:2 * H][:, :, None]
                    .broadcast_to([128, H, DH]))
                nc.vector.tensor_mul(
                    s_t[:, H * DH:2 * H * DH]
                    .rearrange("p (h d) -> p h d", h=H),
                    v3,
                    sin_all[:, tt, H:2 * H][:, :, None]
                    .broadcast_to([128, H, DH]))
                nc.any.tensor_copy(s_t[:, 2 * H * DH:2 * H * DH + H],
                                   cos_all[:, tt, H:2 * H])
                nc.any.tensor_copy(s_t[:, 2 * H * DH + H:SCOLS],
                                   sin_all[:, tt, H:2 * H])
                # raw causal cumsum of S (U-matmul) into resident rc;
                # row 127 = block total feeds the carry exchange
                b, j = tt // 2, tt % 2
                for c0 in range(0, SCOLS, 1024):
                    cw = min(1024, SCOLS - c0)
                    cum = psB.tile([128, 1024], F32, tag="cum", bufs=2)
                    for cc in range(0, cw, 512):
                        ccw = min(512, cw - cc)
                        nc.tensor.matmul(
                            cum[:, cc:cc + ccw], u_sb[:],
                            s_t[:, c0 + cc:c0 + cc + ccw],
                            start=True, stop=True)
                    nc.scalar.copy(out=rc_ts[tt][:, c0:c0 + cw],
                                   in_=cum[:, :cw])
                    nc.sync.dma_start(
                        ag_ins[b][j:j + 1, c0:c0 + cw],
                        rc_ts[tt][127:128, c0:c0 + cw])
                # fire this batch's tiny AllGather as soon as both its
                # tiles are done -- transfers hide under remaining B3
                if j == 1:
                    if skip_collective:
                        nc.gpsimd.dma_start(ag_outs[b][0], ag_ins[b][:])
                    else:
                        nc.gpsimd.collective_compute(
                            "AllGather", mybir.AluOpType.bypass,
                            replica_groups=[list(range(NCORES))],
                            ins=[ag_ins[b].opt()], outs=[ag_outs[b].opt()])

        # ========== phase D: attention + residual + LN2 (per batch) =====
        _mark(nc, phase_marks, 'D_attn')
        sc('D_attn')
        with ExitStack() as pd:
            work = pd.enter_context(tc.tile_pool(name="workD", bufs=3))
            wo_pool = pd.enter_context(tc.tile_pool(name="wop", bufs=1))
            den_all = wo_pool.tile([128, NTT, H], F32, tag="den_all")
            rqc_all = wo_pool.tile([128, NTT, H], F32, tag="rqc_all")
            rqs_all = wo_pool.tile([128, NTT, H], F32, tag="rqs_all")
            psD = pd.enter_context(
                tc.tile_pool(name="psD", bufs=1, space="PSUM"))
            for b in range(B):
                t0 = 2 * b
                # --- D1: scalar-sum carries + denominators for batch b ---
                csc = psD.tile([128, 2, 2 * H], F32, tag="csc", bufs=1)
                gsc = work.tile([2 * NCORES, 2 * H], BF16, tag="gsc", bufs=2)
                nc.sync.dma_start(
                    gsc[:], ag_outs[b][:, :, 2 * H * DH:SCOLS])
                for j in range(2):
                    nc.tensor.matmul(csc[:, j, :],
                                     maskrep[:, j * 128:(j + 1) * 128],
                                     gsc[:], start=True, stop=False)
                    nc.tensor.matmul(csc[:, j, :], eye_sb[:],
                                     rc_ts[t0 + j][:, 2 * H * DH:SCOLS],
                                     start=False, stop=True)
                den = den_all[:, t0:t0 + 2, :]
                t2 = work.tile([128, 2, H], F32, tag="t2", bufs=2)
                nc.vector.tensor_mul(den, csc[:, :, 0:H],
                                     cos_all[:, t0:t0 + 2, 0:H])
                nc.vector.tensor_mul(t2[:], csc[:, :, H:2 * H],
                                     sin_all[:, t0:t0 + 2, 0:H])
                nc.vector.tensor_add(den, den, t2[:])
                nc.vector.tensor_scalar(
                    out=den, in0=den, scalar1=cose_t[:],
                    scalar2=None, op0=mybir.AluOpType.add)
                nc.vector.reciprocal(den, den)
                nc.vector.tensor_mul(rqc_all[:, t0:t0 + 2, :], den,
                                     cos_all[:, t0:t0 + 2, 0:H])
                nc.vector.tensor_mul(rqs_all[:, t0:t0 + 2, :], den,
                                     sin_all[:, t0:t0 + 2, 0:H])

                # --- D2: heads, Wo, residual, LN2 for tiles 2b, 2b+1 ---
                for j in range(2):
                    tt = t0 + j
                    rc_t = rc_ts[tt]
                    gath = work.tile([2 * NCORES, 2 * H * DH], BF16,
                                     tag="gath", bufs=2)
                    nc.sync.dma_start(
                        gath[:], ag_outs[b][:, :, 0:2 * H * DH])

                    h_t = work.tile([128, H * DH], BF16, tag="h_t", bufs=2)
                    tmpc = work.tile([128, H * DH], BF16, tag="tmpc", bufs=2)
                    for half, rqa in ((0, rqc_all), (1, rqs_all)):
                        base = half * H * DH
                        dst = tmpc if half == 0 else h_t
                        for c0 in range(0, H * DH, 512):
                            cv = psD.tile([128, 512], F32, tag="cumv",
                                          bufs=2)
                            nc.tensor.matmul(
                                cv[:],
                                maskrep[:, j * 128:(j + 1) * 128],
                                gath[:, base + c0:base + c0 + 512],
                                start=True, stop=False)
                            nc.tensor.matmul(
                                cv[:], eye_sb[:],
                                rc_t[:, base + c0:base + c0 + 512],
                                start=False, stop=True)
                            nc.vector.tensor_mul(
                                dst[:, c0:c0 + 512]
                                .rearrange("p (h d) -> p h d", h=H // 2),
                                cv[:].rearrange("p (h d) -> p h d", h=H // 2),
                                rqa[:, tt, c0 // DH:(c0 + 512) // DH]
                                [:, :, None].broadcast_to([128, H // 2, DH]))

                    # transpose heads (summing both halves in PSUM) + Wo
                    x_t = work.tile([128, D], F32, tag="x_t2", bufs=2)
                    nc.sync.dma_start(x_t[:],
                                      xs_d[tt * 128:(tt + 1) * 128, :])
                    attn = psD.tile([128, D], F32, tag="attn", bufs=1)
                    for dk in range(NDK):
                        trp = psD.tile([128, 128], F32, tag="trD", bufs=2)
                        nc.tensor.matmul(
                            trp[:], tmpc[:, dk * 128:(dk + 1) * 128],
                            eye_sb[:], start=True, stop=False)
                        nc.tensor.matmul(
                            trp[:], h_t[:, dk * 128:(dk + 1) * 128],
                            eye_sb[:], start=False, stop=True)
                        hT = work.tile([128, 128], BF16, tag="hT", bufs=2)
                        nc.any.tensor_copy(hT[:], trp[:])
                        for nh in range(2):
                            nc.tensor.matmul(
                                attn[:, nh * 512:(nh + 1) * 512], hT[:],
                                wo_sb[:, dk, nh * 512:(nh + 1) * 512],
                                start=(dk == 0), stop=(dk == NDK - 1))
                    # residual add on DVE (PSUM + SBUF -> SBUF)
                    nc.vector.tensor_add(x2ws[tt][:], attn[:], x_t[:])

                    # LN2 + transpose into resident fp8 y2T
                    y_t = work.tile([128, D], FP8, tag="y2_t", bufs=2)
                    _layernorm(nc, nc.vector, work, x2ws[tt], y_t, eps_t,
                               lnw_sb, 2)
                    for dk in range(NDK):
                        trp = psD.tile([128, 128], F32, tag="trD", bufs=2)
                        nc.tensor.matmul(trp[:],
                                         y_t[:, dk * 128:(dk + 1) * 128],
                                         eye8[:], start=True, stop=True)
                        dst = y2T[dk // 2][:, dk % 2,
                                           tt * 128:(tt + 1) * 128]
                        if dk % 2 == 0:
                            nc.vector.tensor_copy(dst, trp[:])
                        else:
                            nc.scalar.copy(out=dst, in_=trp[:])

        rc_stack.close()

        # ================= phase E: MLP (fp8 DoubleRow) =================
        _mark(nc, phase_marks, 'E_mlp')
        sc('E_mlp')
        with ExitStack() as pe:
            wpool = pe.enter_context(tc.tile_pool(name="wmlp", bufs=3))
            h1_pool = pe.enter_context(tc.tile_pool(name="h1p", bufs=1))
            opool = pe.enter_context(tc.tile_pool(name="outp", bufs=3))
            h1 = h1_pool.tile([128, NMP, 2, NTOK], FP8, tag="h1")
            with tc.tile_pool(name="psE1", bufs=1, space="PSUM") as psE1:
                for mt in range(NMT):
                    w1_t = wpool.tile([128, NDP, 2, 128], FP8, tag="w1t")
                    nc.sync.dma_start(w1_t[:], w1_d[:, mt])
                    h1ps = psE1.tile([128, NTOK], F32, tag="h1ps", bufs=2)
                    for dp in range(NDP):
                        for nh in range(2):
                            nc.tensor.matmul(
                                h1ps[:, nh * 512:(nh + 1) * 512],
                                w1_t[:, dp], y2T[dp][:, :,
                                                     nh * 512:(nh + 1) * 512],
                                start=(dp == 0), stop=(dp == NDP - 1),
                                perf_mode=DR)
                    nc.scalar.activation(
                        out=h1[:, mt // 2, mt % 2, :], in_=h1ps[:],
                        func=AF.Gelu_apprx_tanh,
                        bias=b1_sb[:, mt:mt + 1], scale=1.0 / W1S)
            _mark(nc, phase_marks, 'E2_mlp2')
            sc('E2_mlp2')
            with tc.tile_pool(name="psE2", bufs=1, space="PSUM") as psE2:
                for dhalf in range(2):
                    d0 = dhalf * 512
                    ops = [psE2.tile([128, 512], F32, tag=f"o{tt}",
                                     name=f"ops{tt}") for tt in range(NTT)]
                    for mp in range(NMP):
                        w2_t = wpool.tile([128, 2, 512], FP8, tag="w2t",
                                          bufs=4)
                        nc.sync.dma_start(
                            w2_t[:], w2_d[:, mp, :, d0:d0 + 512])
                        for tt in range(NTT):
                            nc.tensor.matmul(
                                ops[tt][:],
                                h1[:, mp, :, tt * 128:(tt + 1) * 128],
                                w2_t[:],
                                start=(mp == 0), stop=False, perf_mode=DR)
                    for tt in range(NTT):
                        nc.tensor.matmul(ops[tt][:], ones1[:],
                                         b2_sb[:, d0:d0 + 512],
                                         start=False, stop=True)
                        # descale W2S on the (idle) scalar engine
                        osc = opool.tile([128, 512], F32, tag="osc")
                        nc.scalar.activation(out=osc[:], in_=ops[tt][:],
                                             func=AF.Copy, scale=1.0 / W2S)
                        o_t = opool.tile([128, 512], F32, tag="o_t")
                        nc.vector.tensor_add(o_t[:], osc[:],
                                             x2ws[tt][:, d0:d0 + 512])
                        nc.sync.dma_start(
                            out_d[tt * 128:(tt + 1) * 128, d0:d0 + 512],
                            o_t[:])
        sc(None)


# ---------------------------------------------------------------------------
# host side
# ---------------------------------------------------------------------------

def _prep_inputs(x, W_Q, W_K, W_V, W_O, ln1_w, ln1_b, ln2_w, ln2_b,
                 W1, b1, W2, b2):
    import ml_dtypes
    f = np.float32
    bf = ml_dtypes.bfloat16
    f8 = ml_dtypes.float8_e4m3
    wqk = np.concatenate(
        [np.asarray(W_Q)[:, :, 0].T, np.asarray(W_K)[:, :, 0].T],
        axis=1).astype(bf)                                       # [D, 2H]
    wv = np.ascontiguousarray(
        np.asarray(W_V).transpose(1, 0, 2).reshape(D, H * DH)).astype(bf)
    wo = np.ascontiguousarray(
        np.asarray(W_O).transpose(2, 1, 0).reshape(H * DH, D)).astype(bf)
    # w1 pre-tiled/paired: w1p8[p, mt, dp, i, c] = W1S*W1[(2dp+i)*128+p, ...]
    w1p8 = np.ascontiguousarray(
        (np.asarray(W1, f) * W1S).reshape(NDP, 2, 128, NMT, 128)
        .transpose(2, 3, 0, 1, 4)).astype(f8)        # [128, NMT, NDP, 2, 128]
    # w2 paired: w2p8[p, mp, i, d] = W2S*W2[(2mp+i)*128+p, d]
    w2p8 = np.ascontiguousarray(
        (np.asarray(W2, f) * W2S).reshape(NMP, 2, 128, D)
        .transpose(2, 0, 1, 3)).astype(f8)           # [128, NMP, 2, D]
    b1r = np.ascontiguousarray(np.asarray(b1, f).reshape(NMT, 128).T)
    b2r = (np.asarray(b2, f) * W2S).reshape(1, D).astype(bf)
    utri = np.triu(np.ones((128, 128), f)).astype(bf)
    eye = np.eye(128, dtype=f).astype(bf)
    eye8 = np.eye(128, dtype=f).astype(f8)
    ones1 = np.ones((1, 128), bf)
    lnw = np.stack([np.asarray(ln1_w, f), np.asarray(ln1_b, f),
                    np.asarray(ln2_w, f), np.asarray(ln2_b, f)])
    common = dict(wqk=wqk, wv=wv, wo=wo,
                  w1p8=w1p8, w2p8=w2p8,
                  b1r=b1r, b2r=b2r, utri=utri, eye=eye, eye8=eye8,
                  ones1=ones1, lnw=lnw)
    x = np.asarray(x, f)
    in_maps = []
    for c in range(NCORES):
        xs = np.ascontiguousarray(
            x[:, c * TC:(c + 1) * TC, :].reshape(NTOK, D))
        masks = np.zeros((2 * NCORES, 2), f)
        for cp in range(NCORES):
            for jp in range(2):
                row = 2 * cp + jp
                masks[row, 0] = 1.0 if cp < c else 0.0
                masks[row, 1] = 1.0 if (cp < c or (cp == c and jp == 0)) \
                    else 0.0
        maskrep = np.concatenate(
            [np.repeat(masks[:, jj:jj + 1], 128, axis=1) for jj in range(2)],
            axis=1).astype(bf)
        in_maps.append(dict(common, xs=xs, maskrep=maskrep))
    trivial = bool(np.allclose(ln1_w, 1) and np.allclose(ln2_w, 1)
                   and np.allclose(ln1_b, 0) and np.allclose(ln2_b, 0))
    return in_maps, trivial


_CACHE = {}


def make_runner(nc):
    """Build a reusable jitted callable for this compiled Bass program."""
    import jax
    from jax.sharding import Mesh, PartitionSpec
    from jax.experimental.shard_map import shard_map

    bass2jax.install_neuronx_cc_hook()
    partition_name = (nc.partition_id_tensor.name
                      if nc.partition_id_tensor else None)
    in_names, out_names, out_avals, zero_outs = [], [], [], []
    for alloc in nc.m.functions[0].allocations:
        if not isinstance(alloc, mybir.MemoryLocationSet):
            continue
        name = alloc.memorylocations[0].name
        if alloc.kind == "ExternalInput":
            if name != partition_name:
                in_names.append(name)
        elif alloc.kind == "ExternalOutput":
            out_names.append(name)
            shape = tuple(alloc.tensor_shape)
            dtype = mybir.dt.np(alloc.dtype)
            out_avals.append(jax.core.ShapedArray(shape, dtype))
            zero_outs.append(np.zeros(shape, dtype))
    n_params = len(in_names)
    n_outs = len(out_avals)
    in_names_all = in_names + out_names
    if partition_name is not None:
        in_names_all.append(partition_name)

    def _bodyfn(*args):
        operands = list(args)
        if partition_name is not None:
            operands.append(bass2jax.partition_id_tensor())
        outs = bass2jax._bass_exec_p.bind(
            *operands,
            out_avals=tuple(out_avals),
            in_names=tuple(in_names_all),
            out_names=tuple(out_names),
            lowering_input_output_aliases=(),
            sim_require_finite=True,
            sim_require_nnan=True,
            nc=nc,
        )
        return tuple(outs)

    devices = jax.devices()[:NCORES]
    mesh = Mesh(np.asarray(devices), ("core",))
    sharded = jax.jit(
        shard_map(_bodyfn, mesh=mesh,
                  in_specs=(PartitionSpec("core"),) * (n_params + n_outs),
                  out_specs=(PartitionSpec("core"),) * n_outs,
                  check_rep=False),
        keep_unused=True)

    def run(in_maps):
        concat_in = [
            np.concatenate([np.asarray(m[name]) for m in in_maps], axis=0)
            for name in in_names
        ]
        zeros = [np.zeros((NCORES * z.shape[0], *z.shape[1:]), z.dtype)
                 for z in zero_outs]
        outs = sharded(*concat_in, *zeros)
        jax.block_until_ready(outs)
        return {
            name: np.asarray(outs[i]).reshape(NCORES, *out_avals[i].shape)
            for i, name in enumerate(out_names)
        }

    return run


def kernel(**inputs):
    in_maps, trivial = _prep_inputs(**inputs)
    key = ("prog", trivial)
    if key not in _CACHE:
        _CACHE[key] = build_program(trivial_ln=trivial)
    nc = _CACHE[key]
    rkey = ("run", trivial)
    if rkey not in _CACHE:
        _CACHE[rkey] = make_runner(nc)
    outs = _CACHE[rkey](in_maps)
    out = outs["out"]  # [NCORES, NTOK, D]
    res = np.empty((B, T, D), np.float32)
    for c in range(NCORES):
        res[:, c * TC:(c + 1) * TC, :] = out[c].reshape(B, TC, D)
    return res
